# revision 18
# baseline (speedup 1.0000x reference)
"""Causal multi-head attention (b=2, n=2048, d=768, 12 heads) on 8 TRN2 NeuronCores.

Sharding: batch x head-group. Core c handles batch c//4 and heads 3*(c%4) .. 3*(c%4)+2.
Each core gets xT = x[b].T plus W.T column slices for its 3 heads, computes the
unnormalized attention output (transposed) plus softmax denominators; the host
divides, transposes, and concatenates slabs into the full [2, 2048, 768].

Schedule (all facts HW-measured on this problem):
  - Attention is ONE flattened software-pipelined stream over
    (span, head-pass, j-tile) iterations: score matmuls run 2 j-tiles ahead
    of the AV matmuls (the PE queue is in-order; AV would otherwise stall on
    ACT's exp), and projection/transpose work for later spans interleaves as
    PE filler.  Steady state measures ~94% PE occupancy at ~230ns per
    512-row f32r matmul (~1.08 cyc/row).
  - Spans stay 512 wide, heads split 2+1 across two passes: a 3-head-fused
    256-wide variant measured SLOWER (133us vs 125) because LDWEIGHTS
    (~134-190ns per matmul) no longer hides under a 107ns moving stream.
  - p / vnat stay f32r: ACT exp writing bf16 measured ~35% slower
    (1337 vs 997ns for a [128,2,512] exp), sinking the exp-bound phase.
  - Startup: the engine-init preamble blocks all queues until ~5.5us.
    The const blob (identity/trimask/ones, one small DMA) leads the sync
    queue so PE warmup starts ~6.5us; x span0 follows in half-span chunks
    feeding a kt-outer span-0 projection with 5 open PSUM groups; weights
    ride the scalar queue in 3 chunks.  Any PE gap >3.4us re-throttles the
    HAM clock gate to 1.2GHz, so the filler machinery keeps PE dense.
  - f32r SBUF tiles cannot be memset (and DMAing 2MB of DRAM zeros for the
    packed-head pads serialized the x DMAs ~10us): pads are DVE copies from
    a zeros tile emitted right after each span's k/q copies.
  - Every matmul keeps contraction K=128 (zero-padded kT/qT2): K<128 streams
    drop the HAM clock gate to 1.2 GHz and stop back-to-back pipelining.
"""
import sys

if "/opt/trn_rl_repo" not in sys.path:
    sys.path.insert(0, "/opt/trn_rl_repo")

from collections import deque
from contextlib import ExitStack

import numpy as np

import concourse.bass as bass
import concourse.tile as tile
from concourse import bacc, mybir, bass_utils
from concourse.masks import make_identity, make_upper_triangular

F32 = mybir.dt.float32
F32R = mybir.dt.float32r

P = 128
SPAN = 512
HD = 64

B, N, D, NH = 2, 2048, 768, 12
HL = 3                       # heads per core
DL = HL * HD                 # 192
N_CORES = 8
KT = D // P                  # 6 contraction chunks
NS = N // SPAN               # 4 spans
NT = N // P                  # 16 j-tiles
CPS = SPAN // P              # 4 j-tiles per span
WCW = 3 * DL                 # 576 weight cols per kt chunk
EXP = mybir.ActivationFunctionType.Exp

# m-chunk packing inside wc: [q01 | k01 | v01 | k2+v2 | q2]
M_CHUNKS = ((0, P, "q01"), (P, P, "k01"), (2 * P, P, "v01"),
            (3 * P, P, "k2v2"), (4 * P, HD, "q2"))


def _build(nc, tc):
    xt = nc.dram_tensor("xt", [P, N * KT], F32R, kind="ExternalInput").ap()
    wc = nc.dram_tensor("wc", [P, KT * WCW], F32R, kind="ExternalInput").ap()
    o = nc.dram_tensor("o", [HL * (HD + 1), N], F32, kind="ExternalOutput").ap()

    with ExitStack() as ctx:
        pool = lambda name, bufs, **kw: ctx.enter_context(
            tc.tile_pool(name=name, bufs=bufs, **kw))
        const_pool = pool("const", 1)
        xpool = pool("x", NS)
        wpool = pool("w", 1)
        qpool = pool("q", 1)
        vtpool = pool("vt", 1)
        kzpool = pool("kz", 1)
        vnat_pool = pool("vnat", 1)
        ppool = pool("p", 4)
        osb_pool = pool("osb", 3)

        # ---- SBUF tiles ----
        x_tiles = [xpool.tile([P, KT * SPAN], F32R, tag="x", name=f"x{i}")
                   for i in range(NS)]
        w_all = wpool.tile([P, KT * WCW], F32R, tag="w", name="w_all")
        qT01 = qpool.tile([P, N], F32R, tag="q01", name="qT01")
        qT2z = qpool.tile([P, N], F32R, tag="q2z", name="qT2z")
        vT01 = vtpool.tile([P, N], F32, tag="v01", name="vT01")
        vT2z = vtpool.tile([P, N], F32, tag="v2z", name="vT2z")
        kTz = [kzpool.tile([P, N], F32R, tag=f"kz{h}", name=f"kTz{h}")
               for h in range(HL)]
        v_nat01 = vnat_pool.tile([P, NT * 2 * (HD + 1)], F32R, tag="vnat01",
                                 name="v_nat01")
        v_nat2 = vnat_pool.tile([P, NT * (HD + 1)], F32R, tag="vnat2",
                                name="v_nat2")

        ident_t = const_pool.tile([P, P], F32, name="ident_t")
        trimask_t = const_pool.tile([P, P], F32, name="trimask_t")
        ones_t = const_pool.tile([P, 2 * NT], F32, name="ones_t")
        zeros = const_pool.tile([P, SPAN], F32, name="zeros")
        scratch = const_pool.tile([1, 8], F32, name="scratch")
        ident = ident_t[:]
        trimask = trimask_t[:]
        ones_f = ones_t[:]

        def x_slice(ns, kt):
            return x_tiles[ns][:, kt * SPAN:(kt + 1) * SPAN]

        def w_slice(kt, c0, c1):
            b = kt * WCW
            return w_all[:, b + c0:b + c1]

        def vnat_lhsT(h, jt):
            if h < 2:
                b = jt * 2 * (HD + 1) + h * (HD + 1)
                return v_nat01[:, b:b + HD + 1]
            b = jt * (HD + 1)
            return v_nat2[:, b:b + HD + 1]

        # ---- DMAs ----
        # sync queue: x span0 in half-span chunks (kt-outer projection starts
        # after the first), then the remaining spans.
        for kt in range(KT):
            nc.sync.dma_start(x_slice(0, kt), xt[:, kt * SPAN:(kt + 1) * SPAN])
        for ns in range(1, NS):
            w = KT * SPAN
            nc.sync.dma_start(x_tiles[ns][:], xt[:, ns * w:(ns + 1) * w])
        # scalar queue: weights per kt chunk (small first chunk arrives
        # before the projection needs it)
        for kt in range(KT):
            nc.scalar.dma_start(
                w_all[:, kt * WCW:(kt + 1) * WCW],
                wc[:, kt * WCW:(kt + 1) * WCW])

        # ---- gpsimd-generated consts (ident first: warmup needs it) ----
        make_identity(nc, ident)
        nc.gpsimd.memset(zeros[:], 0.0)
        make_upper_triangular(nc, trimask, val=1.0, diag=True)
        nc.gpsimd.memset(ones_f, 1.0)
        nc.gpsimd.memset(vT2z[0:HD, :], 0.0)
        c01 = v_nat01[:].rearrange("p (t c) -> p t c", c=HD + 1)[:, :, HD]
        c2 = v_nat2[:].rearrange("p (t c) -> p t c", c=HD + 1)[:, :, HD]
        nc.vector.tensor_copy(c01, ones_f)
        nc.vector.tensor_copy(c2, ones_f[:, 0:NT])
        # span-0 zero pads early on DVE (they gate att(0)'s first scores and
        # would otherwise queue behind the span-0 projection copies)
        nc.vector.tensor_copy(kTz[0][HD:P, 0:SPAN], zeros[0:HD, :])
        nc.vector.tensor_copy(kTz[1][0:HD, 0:SPAN], zeros[0:HD, :])
        nc.vector.tensor_copy(kTz[2][HD:P, 0:SPAN], zeros[0:HD, :])
        nc.vector.tensor_copy(qT2z[HD:P, 0:SPAN], zeros[0:HD, :])
        # anchor: loads the ACT exp table during the projection phase
        nc.scalar.activation(scratch[0:1, 0:1], ident[0:1, 0:1], EXP)
        # ---- projection copies (psum group -> packed sbuf tiles), plus the
        # per-span f32r zero pads for the unused head-halves ----
        def emit_copies(what, g, s):
            c0 = s * SPAN
            pads = s >= 1
            if what == "q01":
                nc.vector.tensor_copy(qT01[:, c0:c0 + SPAN], g[:])
            elif what == "k01":
                nc.vector.tensor_copy(kTz[0][0:HD, c0:c0 + SPAN], g[0:HD, :])
                nc.vector.tensor_copy(kTz[1][HD:P, c0:c0 + SPAN], g[HD:P, :])
                if pads:
                    nc.vector.tensor_copy(kTz[0][HD:P, c0:c0 + SPAN],
                                          zeros[0:HD, :])
                    nc.vector.tensor_copy(kTz[1][0:HD, c0:c0 + SPAN],
                                          zeros[0:HD, :])
            elif what == "v01":
                nc.vector.tensor_copy(vT01[:, c0:c0 + SPAN], g[:])
            elif what == "k2v2":
                nc.vector.tensor_copy(kTz[2][0:HD, c0:c0 + SPAN], g[0:HD, :])
                nc.vector.tensor_copy(vT2z[HD:P, c0:c0 + SPAN], g[HD:P, :])
                if pads:
                    nc.vector.tensor_copy(kTz[2][HD:P, c0:c0 + SPAN],
                                          zeros[0:HD, :])
            else:  # q2
                nc.vector.tensor_copy(qT2z[0:HD, c0:c0 + SPAN], g[:])
                if pads:
                    nc.vector.tensor_copy(qT2z[HD:P, c0:c0 + SPAN],
                                          zeros[0:HD, :])

        holder = {}

        def emit_tp(s, c, pl=None, tag="pa"):
            # v01 natural-layout transpose for j-tile jt = 4*s + c
            jt = s * CPS + c
            tp = (pl or holder["pa"]).tile([P, P], F32, tag=tag,
                                           name=f"tp_{jt}")
            nc.tensor.transpose(tp[:], vT01[:, jt * P:(jt + 1) * P], ident)
            nc.vector.tensor_copy(
                v_nat01[:].rearrange("p (t c) -> p t c", c=HD + 1)[
                    :, 2 * jt:2 * jt + 2, 0:HD],
                tp[:].rearrange("p (t c) -> p t c", c=HD))

        def emit_tp2(s, c, pl=None, tag="pa"):
            jt = s * CPS + c
            tp2 = (pl or holder["pa"]).tile([P, P], F32, tag=tag,
                                            name=f"tp2_{jt}")
            nc.tensor.transpose(tp2[:], vT2z[:, jt * P:(jt + 1) * P], ident)
            nc.vector.tensor_copy(
                v_nat2[:, jt * (HD + 1):jt * (HD + 1) + HD], tp2[:, HD:P])

        # ---- phase 0: warmup + span-0 projection.  q01+k01 run kt-outer
        # (paced by the x0 kt-chunk DMAs) and close first so their DVE
        # copies overlap the remaining m-outer groups; att(0)'s first score
        # then only waits on q01/k01.  v transposes become att(0) fillers.
        with tc.tile_pool(name="pg", bufs=3, space="PSUM") as pg:
            warm = pg.tile([P, SPAN], F32, tag="pg", name="warm")
            for _ in range(12):
                nc.tensor.matmul(warm[:, 0:P], ident, ident,
                                 start=True, stop=True)
            gA = [pg.tile([msz, SPAN], F32, tag="pg", name=f"g_{what}")
                  for (_, msz, what) in M_CHUNKS[:2]]
            for kt in range(KT):
                for ci, (moff, msz, what) in enumerate(M_CHUNKS[:2]):
                    nc.tensor.matmul(gA[ci][:], w_slice(kt, moff, moff + msz),
                                     x_slice(0, kt),
                                     start=(kt == 0), stop=(kt == KT - 1))
            for ci, (_, _, what) in enumerate(M_CHUNKS[:2]):
                emit_copies(what, gA[ci], 0)
            for (moff, msz, what) in M_CHUNKS[2:]:
                gB = pg.tile([msz, SPAN], F32, tag="pg", name=f"g_{what}")
                for kt in range(KT):
                    nc.tensor.matmul(gB[:], w_slice(kt, moff, moff + msz),
                                     x_slice(0, kt),
                                     start=(kt == 0), stop=(kt == KT - 1))
                emit_copies(what, gB, 0)

        # ---- attention-phase psum pools: 4 + 2 + 2 banks = 8 ----
        ps_sc = pool("ps_sc", 2, space="PSUM")
        ps_av = pool("ps_av", 2, space="PSUM")
        ps_pa = pool("ps_pa", 2, space="PSUM")
        holder["pa"] = ps_pa

        # ---- filler units: projections + v transposes for spans 1..3 ----
        def make_group_units(s, chunks, state):
            units = []
            for (moff, msz, what) in chunks:
                for kt in range(KT):
                    def u(kt=kt, moff=moff, msz=msz, what=what):
                        if kt == 0:
                            state[what] = ps_pa.tile(
                                [msz, SPAN], F32, tag="pa", name=f"g{s}_{what}")
                        nc.tensor.matmul(
                            state[what][:], w_slice(kt, moff, moff + msz),
                            x_slice(s, kt),
                            start=(kt == 0), stop=(kt == KT - 1))
                        if kt == KT - 1:
                            emit_copies(what, state[what], s)
                    units.append(u)
            return units

        def make_early_units(s):
            # q01 / k01 / v01 groups (needed before att(s) h01)
            return make_group_units(s, M_CHUNKS[:3], {})

        def make_late_units(s):
            # tp first (vnat01 slots 4s.. are read from av(4s), emitted 2+
            # iterations into the pass), then k2v2 (writes vT2z) before tp2,
            # then q2.  All of late(s) drains before att(s) h2.
            state = {}
            units = [(lambda c=c: emit_tp(s, c)) for c in range(CPS)]
            units += make_group_units(s, (M_CHUNKS[3],), state)
            units += [(lambda c=c: emit_tp2(s, c)) for c in range(CPS)]
            units += make_group_units(s, (M_CHUNKS[4],), state)
            return units

        # filler queue with barriers: late(s) fills att(s) h01, drains at the
        # (s,1) barrier; early(s+1) fills att(s) h2, drains at (s+1,0).
        fq = deque()
        for s in range(NS):
            fq.append(("barrier", (s, 0)))
            fq.extend(make_late_units(s))
            fq.append(("barrier", (s, 1)))
            if s + 1 < NS:
                fq.extend(make_early_units(s + 1))

        def drain_until(key):
            while fq:
                u = fq.popleft()
                if isinstance(u, tuple) and u[0] == "barrier":
                    if u[1] == key:
                        return
                    continue
                u()

        def pop_filler():
            if fq and not (isinstance(fq[0], tuple) and fq[0][0] == "barrier"):
                fq.popleft()()

        # ---- flattened, software-pipelined attention ----
        avq = deque()

        def pump(force=False):
            while avq and (force or len(avq) > 2):
                av, fin = avq.popleft()
                av()
                if fin:
                    fin()

        def finalize(h, s, av):
            def f():
                ob = osb_pool.tile([HD + 1, SPAN], F32, tag="osb",
                                   name=f"ob{h}_{s}")
                nc.vector.tensor_copy(ob[:], av[:])
                nc.gpsimd.dma_start(
                    o[h * (HD + 1):(h + 1) * (HD + 1),
                      s * SPAN:(s + 1) * SPAN], ob[:])
            return f

        for s in range(NS):
            njt = CPS * (s + 1)
            q0 = s * SPAN

            # ---- h01 pass ----
            drain_until((s, 0))
            av0 = ps_av.tile([HD + 1, SPAN], F32, tag="av", name=f"av0_{s}")
            av1 = ps_av.tile([HD + 1, SPAN], F32, tag="av", name=f"av1_{s}")
            for jt in range(njt):
                c_d = jt - CPS * s
                n0 = max(c_d, 0) * P
                sc = ps_sc.tile([P, 2 * SPAN], F32, tag="sc",
                                name=f"sc_{s}_{jt}")
                nc.tensor.matmul(sc[:, n0:SPAN],
                                 kTz[0][:, jt * P:(jt + 1) * P],
                                 qT01[:, q0 + n0:q0 + SPAN],
                                 start=True, stop=True)
                nc.tensor.matmul(sc[:, SPAN + n0:2 * SPAN],
                                 kTz[1][:, jt * P:(jt + 1) * P],
                                 qT01[:, q0 + n0:q0 + SPAN],
                                 start=True, stop=True)
                pop_filler()
                if 0 <= c_d <= 1:
                    pop_filler()
                p = ppool.tile([P, 2 * SPAN], F32R, tag="p", name=f"p_{s}_{jt}")
                if n0 == 0:
                    nc.scalar.activation(p[:], sc[:], EXP)
                else:
                    sc3 = sc[:].rearrange("q (t c) -> q t c", c=SPAN)
                    p3 = p[:].rearrange("q (t c) -> q t c", c=SPAN)
                    nc.scalar.activation(p3[:, :, n0:SPAN], sc3[:, :, n0:SPAN],
                                         EXP)
                if c_d >= 0:
                    nc.vector.tensor_mul(
                        p[:, n0:n0 + P], p[:, n0:n0 + P], trimask)
                    nc.vector.tensor_mul(
                        p[:, SPAN + n0:SPAN + n0 + P],
                        p[:, SPAN + n0:SPAN + n0 + P], trimask)

                def av_emit(jt=jt, n0=n0, p=p, av0=av0, av1=av1, njt=njt):
                    nc.tensor.matmul(av0[:, n0:SPAN], vnat_lhsT(0, jt),
                                     p[:, n0:SPAN],
                                     start=(jt == 0), stop=(jt == njt - 1))
                    nc.tensor.matmul(av1[:, n0:SPAN], vnat_lhsT(1, jt),
                                     p[:, SPAN + n0:2 * SPAN],
                                     start=(jt == 0), stop=(jt == njt - 1))
                fin = None
                if jt == njt - 1:
                    f0, f1 = finalize(0, s, av0), finalize(1, s, av1)
                    fin = lambda f0=f0, f1=f1: (f0(), f1())
                avq.append((av_emit, fin))
                pump()

            # ---- h2 pass ----
            drain_until((s, 1))
            av2 = ps_av.tile([HD + 1, SPAN], F32, tag="av", name=f"av2_{s}")
            for jt in range(njt):
                c_d = jt - CPS * s
                n0 = max(c_d, 0) * P
                sc = ps_sc.tile([P, 2 * SPAN], F32, tag="sc",
                                name=f"sc2_{s}_{jt}")
                nc.tensor.matmul(sc[:, n0:SPAN],
                                 kTz[2][:, jt * P:(jt + 1) * P],
                                 qT2z[:, q0 + n0:q0 + SPAN],
                                 start=True, stop=True)
                pop_filler()
                if 0 <= c_d <= 1:
                    pop_filler()
                p = ppool.tile([P, 2 * SPAN], F32R, tag="p",
                               name=f"p2_{s}_{jt}")
                nc.scalar.activation(p[:, n0:SPAN], sc[:, n0:SPAN], EXP)
                if c_d >= 0:
                    nc.vector.tensor_mul(
                        p[:, n0:n0 + P], p[:, n0:n0 + P], trimask)

                def av_emit(jt=jt, n0=n0, p=p, av2=av2, njt=njt):
                    nc.tensor.matmul(av2[:, n0:SPAN], vnat_lhsT(2, jt),
                                     p[:, n0:SPAN],
                                     start=(jt == 0), stop=(jt == njt - 1))
                fin = finalize(2, s, av2) if jt == njt - 1 else None
                avq.append((av_emit, fin))
                pump()

        while fq:
            u = fq.popleft()
            if not (isinstance(u, tuple) and u[0] == "barrier"):
                u()
        pump(force=True)


_NC_CACHE = {}


def _get_module():
    key = "v11"
    if key not in _NC_CACHE:
        nc = bacc.Bacc("TRN2", target_bir_lowering=False, debug=False)
        with tile.TileContext(nc) as tc:
            _build(nc, tc)
        nc.compile()
        _NC_CACHE[key] = nc
    return _NC_CACHE[key]



def _in_maps(x, Wq, Wk, Wv):
    maps = []
    xT = [np.ascontiguousarray(
        x[b].T.reshape(KT, P, NS, SPAN).transpose(1, 2, 0, 3).reshape(P, -1))
        for b in range(B)]
    WqT, WkT, WvT = Wq.T, Wk.T, Wv.T
    for c in range(N_CORES):
        bc, g = divmod(c, N_CORES // B)
        s0 = g * DL
        wcomb = np.concatenate([
            WqT[:, s0:s0 + P], WkT[:, s0:s0 + P], WvT[:, s0:s0 + P],
            WkT[:, s0 + P:s0 + DL], WvT[:, s0 + P:s0 + DL],
            WqT[:, s0 + P:s0 + DL]], axis=1)
        wpk = np.ascontiguousarray(
            wcomb.reshape(KT, P, WCW).transpose(1, 0, 2).reshape(P, -1))
        maps.append({
            "xt": xT[bc],
            "wc": wpk,
        })
    return maps


def kernel(x, Wq, Wk, Wv, _trace=False, _tmpdir=None, **_kw):
    x = np.asarray(x, dtype=np.float32)
    Wq = np.asarray(Wq, dtype=np.float32)
    Wk = np.asarray(Wk, dtype=np.float32)
    Wv = np.asarray(Wv, dtype=np.float32)
    assert x.shape == (B, N, D) and Wq.shape == (D, D)

    nc = _get_module()
    res = bass_utils.run_bass_kernel_spmd(
        nc, _in_maps(x, Wq, Wk, Wv), core_ids=list(range(N_CORES)),
        trace=_trace, tmpdir=_tmpdir)
    out = np.empty((B, N, D), np.float32)
    for c in range(N_CORES):
        bc, g = divmod(c, N_CORES // B)
        oT = res.results[c]["o"].astype(np.float64)
        for h in range(HL):
            blk = oT[h * (HD + 1):h * (HD + 1) + HD, :]
            den = oT[h * (HD + 1) + HD, :]
            out[bc, :, g * DL + h * HD:g * DL + (h + 1) * HD] = \
                (blk / den).T.astype(np.float32)
    if _trace:
        return out, res
    return out


# revision 19
# speedup vs baseline: 1.0686x; 1.0686x over previous
"""Causal multi-head attention (b=2, n=2048, d=768, 12 heads) on 8 TRN2 NeuronCores.

Sharding: batch x head-group. Core c handles batch c//4 and heads 3*(c%4) .. 3*(c%4)+2.
Each core gets xT = x[b].T plus W.T column slices for its 3 heads, computes the
unnormalized attention output (transposed) plus softmax denominators; the host
divides, transposes, and concatenates slabs into the full [2, 2048, 768].

Schedule (all facts HW-measured on this problem):
  - Attention is ONE flattened software-pipelined stream over
    (span, head-pass, j-tile) iterations: score matmuls run 2 j-tiles ahead
    of the AV matmuls (the PE queue is in-order; AV would otherwise stall on
    ACT's exp), and projection/transpose work for later spans interleaves as
    PE filler.  Steady state measures ~94% PE occupancy at ~230ns per
    512-row f32r matmul (~1.08 cyc/row).
  - Spans stay 512 wide, heads split 2+1 across two passes: a 3-head-fused
    256-wide variant measured SLOWER (133us vs 125) because LDWEIGHTS
    (~134-190ns per matmul) no longer hides under a 107ns moving stream.
  - p / vnat stay f32r: ACT exp writing bf16 measured ~35% slower
    (1337 vs 997ns for a [128,2,512] exp), sinking the exp-bound phase.
  - Startup: the engine-init preamble blocks all queues until ~5.5us.
    The const blob (identity/trimask/ones, one small DMA) leads the sync
    queue so PE warmup starts ~6.5us; x span0 follows in half-span chunks
    feeding a kt-outer span-0 projection with 5 open PSUM groups; weights
    ride the scalar queue in 3 chunks.  Any PE gap >3.4us re-throttles the
    HAM clock gate to 1.2GHz, so the filler machinery keeps PE dense.
  - f32r SBUF tiles cannot be memset (and DMAing 2MB of DRAM zeros for the
    packed-head pads serialized the x DMAs ~10us): pads are DVE copies from
    a zeros tile emitted right after each span's k/q copies.
  - Every matmul keeps contraction K=128 (zero-padded kT/qT2): K<128 streams
    drop the HAM clock gate to 1.2 GHz and stop back-to-back pipelining.
"""
import sys

if "/opt/trn_rl_repo" not in sys.path:
    sys.path.insert(0, "/opt/trn_rl_repo")

from collections import deque
from contextlib import ExitStack

import numpy as np

import concourse.bass as bass
import concourse.tile as tile
from concourse import bacc, mybir, bass_utils

F32 = mybir.dt.float32
F32R = mybir.dt.float32r

P = 128
SPAN = 512
HD = 64

B, N, D, NH = 2, 2048, 768, 12
HL = 3                       # heads per core
DL = HL * HD                 # 192
N_CORES = 8
KT = D // P                  # 6 contraction chunks
NS = N // SPAN               # 4 spans
NT = N // P                  # 16 j-tiles
CPS = SPAN // P              # 4 j-tiles per span
WCW = 3 * DL                 # 576 weight cols per kt chunk
EXP = mybir.ActivationFunctionType.Exp

# m-chunk packing inside wc: [q01 | k01 | v01 | k2+v2 | q2]
M_CHUNKS = ((0, P, "q01"), (P, P, "k01"), (2 * P, P, "v01"),
            (3 * P, P, "k2v2"), (4 * P, HD, "q2"))

# const blob layout (cols): [ident 0:128 | trimask 128:256 | ones 256:288]
CZ_W = P + P + 2 * NT


def _build(nc, tc):
    xt = nc.dram_tensor("xt", [P, N * KT], F32R, kind="ExternalInput").ap()
    wc = nc.dram_tensor("wc", [P, KT * WCW], F32R, kind="ExternalInput").ap()
    cz = nc.dram_tensor("cz", [P, CZ_W], F32, kind="ExternalInput").ap()
    o = nc.dram_tensor("o", [HL * (HD + 1), N], F32, kind="ExternalOutput").ap()

    with ExitStack() as ctx:
        pool = lambda name, bufs, **kw: ctx.enter_context(
            tc.tile_pool(name=name, bufs=bufs, **kw))
        const_pool = pool("const", 1)
        xpool = pool("x", NS)
        wpool = pool("w", 1)
        qpool = pool("q", 1)
        vtpool = pool("vt", 1)
        kzpool = pool("kz", 1)
        vnat_pool = pool("vnat", 1)
        ppool = pool("p", 4)
        osb_pool = pool("osb", 3)

        # ---- SBUF tiles ----
        x_tiles = [xpool.tile([P, KT * SPAN], F32R, tag="x", name=f"x{i}")
                   for i in range(NS)]
        w_all = wpool.tile([P, KT * WCW], F32R, tag="w", name="w_all")
        qT01 = qpool.tile([P, N], F32R, tag="q01", name="qT01")
        qT2z = qpool.tile([P, N], F32R, tag="q2z", name="qT2z")
        vT01 = vtpool.tile([P, N], F32, tag="v01", name="vT01")
        vT2z = vtpool.tile([P, N], F32, tag="v2z", name="vT2z")
        kTz = [kzpool.tile([P, N], F32R, tag=f"kz{h}", name=f"kTz{h}")
               for h in range(HL)]
        v_nat01 = vnat_pool.tile([P, NT * 2 * (HD + 1)], F32R, tag="vnat01",
                                 name="v_nat01")
        v_nat2 = vnat_pool.tile([P, NT * (HD + 1)], F32R, tag="vnat2",
                                name="v_nat2")

        czt = const_pool.tile([P, CZ_W], F32, name="czt")
        ident = czt[:, 0:P]
        trimask = czt[:, P:2 * P]
        ones_f = czt[:, 2 * P:CZ_W]
        zeros = const_pool.tile([P, SPAN], F32, name="zeros")
        scratch = const_pool.tile([1, 8], F32, name="scratch")

        def x_slice(ns, kt):
            return x_tiles[ns][:, kt * SPAN:(kt + 1) * SPAN]

        def w_slice(kt, c0, c1):
            b = kt * WCW
            return w_all[:, b + c0:b + c1]

        def vnat_lhsT(h, jt):
            if h < 2:
                b = jt * 2 * (HD + 1) + h * (HD + 1)
                return v_nat01[:, b:b + HD + 1]
            b = jt * (HD + 1)
            return v_nat2[:, b:b + HD + 1]

        # ---- DMAs ----
        # sync queue: consts first (tiny), then x span0 in half-span chunks,
        # then the remaining spans.
        nc.sync.dma_start(czt[:], cz[:, :])
        HK = KT // 2
        for h in range(2):
            nc.sync.dma_start(
                x_tiles[0][:, h * HK * SPAN:(h + 1) * HK * SPAN],
                xt[:, h * HK * SPAN:(h + 1) * HK * SPAN])
        for ns in range(1, NS):
            w = KT * SPAN
            nc.sync.dma_start(x_tiles[ns][:], xt[:, ns * w:(ns + 1) * w])
        # scalar queue: weights in 3 chunks of 2 kt
        for h in range(3):
            nc.scalar.dma_start(
                w_all[:, h * 2 * WCW:(h + 1) * 2 * WCW],
                wc[:, h * 2 * WCW:(h + 1) * 2 * WCW])

        # ---- gpsimd/DVE-generated pieces ----
        nc.gpsimd.memset(zeros[:], 0.0)
        nc.gpsimd.memset(vT2z[0:HD, :], 0.0)
        c01 = v_nat01[:].rearrange("p (t c) -> p t c", c=HD + 1)[:, :, HD]
        c2 = v_nat2[:].rearrange("p (t c) -> p t c", c=HD + 1)[:, :, HD]
        nc.vector.tensor_copy(c01, ones_f)
        nc.vector.tensor_copy(c2, ones_f[:, 0:NT])
        # span-0 zero pads early on DVE (they gate att(0)'s first scores and
        # would otherwise queue behind the span-0 projection copies)
        nc.vector.tensor_copy(kTz[0][HD:P, 0:SPAN], zeros[0:HD, :])
        nc.vector.tensor_copy(kTz[1][0:HD, 0:SPAN], zeros[0:HD, :])
        nc.vector.tensor_copy(kTz[2][HD:P, 0:SPAN], zeros[0:HD, :])
        nc.vector.tensor_copy(qT2z[HD:P, 0:SPAN], zeros[0:HD, :])
        # anchor: loads the ACT exp table during the projection phase
        nc.scalar.activation(scratch[0:1, 0:1], ident[0:1, 0:1], EXP)
        # ---- projection copies (psum group -> packed sbuf tiles), plus the
        # per-span f32r zero pads for the unused head-halves ----
        def emit_copies(what, g, s):
            c0 = s * SPAN
            pads = s >= 1
            if what == "q01":
                nc.vector.tensor_copy(qT01[:, c0:c0 + SPAN], g[:])
            elif what == "k01":
                nc.vector.tensor_copy(kTz[0][0:HD, c0:c0 + SPAN], g[0:HD, :])
                nc.vector.tensor_copy(kTz[1][HD:P, c0:c0 + SPAN], g[HD:P, :])
                if pads:
                    nc.vector.tensor_copy(kTz[0][HD:P, c0:c0 + SPAN],
                                          zeros[0:HD, :])
                    nc.vector.tensor_copy(kTz[1][0:HD, c0:c0 + SPAN],
                                          zeros[0:HD, :])
            elif what == "v01":
                nc.vector.tensor_copy(vT01[:, c0:c0 + SPAN], g[:])
            elif what == "k2v2":
                nc.vector.tensor_copy(kTz[2][0:HD, c0:c0 + SPAN], g[0:HD, :])
                nc.vector.tensor_copy(vT2z[HD:P, c0:c0 + SPAN], g[HD:P, :])
                if pads:
                    nc.vector.tensor_copy(kTz[2][HD:P, c0:c0 + SPAN],
                                          zeros[0:HD, :])
            else:  # q2
                nc.vector.tensor_copy(qT2z[0:HD, c0:c0 + SPAN], g[:])
                if pads:
                    nc.vector.tensor_copy(qT2z[HD:P, c0:c0 + SPAN],
                                          zeros[0:HD, :])

        holder = {}

        def emit_tp(s, c, pl=None, tag="pa"):
            # v01 natural-layout transpose for j-tile jt = 4*s + c
            jt = s * CPS + c
            tp = (pl or holder["pa"]).tile([P, P], F32, tag=tag,
                                           name=f"tp_{jt}")
            nc.tensor.transpose(tp[:], vT01[:, jt * P:(jt + 1) * P], ident)
            nc.vector.tensor_copy(
                v_nat01[:].rearrange("p (t c) -> p t c", c=HD + 1)[
                    :, 2 * jt:2 * jt + 2, 0:HD],
                tp[:].rearrange("p (t c) -> p t c", c=HD))

        def emit_tp2(s, c, pl=None, tag="pa"):
            jt = s * CPS + c
            tp2 = (pl or holder["pa"]).tile([P, P], F32, tag=tag,
                                            name=f"tp2_{jt}")
            nc.tensor.transpose(tp2[:], vT2z[:, jt * P:(jt + 1) * P], ident)
            nc.vector.tensor_copy(
                v_nat2[:, jt * (HD + 1):jt * (HD + 1) + HD], tp2[:, HD:P])

        # ---- phase 0: warmup + span-0 projection (kt-outer, 5 open psum
        # groups) + span-0 transposes ----
        with tc.tile_pool(name="pg", bufs=5, space="PSUM") as pg:
            warm = pg.tile([P, SPAN], F32, tag="pg", name="warm")
            for _ in range(10):
                nc.tensor.matmul(warm[:, 0:P], ident, ident,
                                 start=True, stop=True)
            g0 = [pg.tile([msz, SPAN], F32, tag="pg", name=f"g_{what}")
                  for (_, msz, what) in M_CHUNKS]
            for kt in range(KT):
                for ci, (moff, msz, what) in enumerate(M_CHUNKS):
                    nc.tensor.matmul(g0[ci][:], w_slice(kt, moff, moff + msz),
                                     x_slice(0, kt),
                                     start=(kt == 0), stop=(kt == KT - 1))
            for ci, (_, _, what) in enumerate(M_CHUNKS):
                emit_copies(what, g0[ci], 0)
            for c in range(CPS):
                emit_tp(0, c, pl=pg, tag="pg")
            for c in range(CPS):
                emit_tp2(0, c, pl=pg, tag="pg")

        # ---- attention-phase psum pools: 4 + 2 + 2 banks = 8 ----
        ps_sc = pool("ps_sc", 2, space="PSUM")
        ps_av = pool("ps_av", 2, space="PSUM")
        ps_pa = pool("ps_pa", 2, space="PSUM")
        holder["pa"] = ps_pa

        # ---- filler units: projections + v transposes for spans 1..3 ----
        def make_group_units(s, chunks, state):
            units = []
            for (moff, msz, what) in chunks:
                for kt in range(KT):
                    def u(kt=kt, moff=moff, msz=msz, what=what):
                        if kt == 0:
                            state[what] = ps_pa.tile(
                                [msz, SPAN], F32, tag="pa", name=f"g{s}_{what}")
                        nc.tensor.matmul(
                            state[what][:], w_slice(kt, moff, moff + msz),
                            x_slice(s, kt),
                            start=(kt == 0), stop=(kt == KT - 1))
                        if kt == KT - 1:
                            emit_copies(what, state[what], s)
                    units.append(u)
            return units

        def make_early_units(s):
            # q01 / k01 / v01 groups (needed before att(s) h01)
            return make_group_units(s, M_CHUNKS[:3], {})

        def make_late_units(s):
            # tp first (vnat01 slots 4s.. are read from av(4s), emitted 2+
            # iterations into the pass), then k2v2 (writes vT2z) before tp2,
            # then q2.  All of late(s) drains before att(s) h2.
            state = {}
            units = [(lambda c=c: emit_tp(s, c)) for c in range(CPS)]
            units += make_group_units(s, (M_CHUNKS[3],), state)
            units += [(lambda c=c: emit_tp2(s, c)) for c in range(CPS)]
            units += make_group_units(s, (M_CHUNKS[4],), state)
            return units

        # filler queue with barriers: late(s) fills att(s) h01, drains at the
        # (s,1) barrier; early(s+1) fills att(s) h2, drains at (s+1,0).
        fq = deque()
        for s in range(NS):
            fq.append(("barrier", (s, 0)))
            if s >= 1:
                fq.extend(make_late_units(s))
            fq.append(("barrier", (s, 1)))
            if s + 1 < NS:
                fq.extend(make_early_units(s + 1))

        def drain_until(key):
            while fq:
                u = fq.popleft()
                if isinstance(u, tuple) and u[0] == "barrier":
                    if u[1] == key:
                        return
                    continue
                u()

        def pop_filler():
            if fq and not (isinstance(fq[0], tuple) and fq[0][0] == "barrier"):
                fq.popleft()()

        # ---- flattened, software-pipelined attention ----
        avq = deque()

        def pump(force=False):
            while avq and (force or len(avq) > 2):
                av, fin = avq.popleft()
                av()
                if fin:
                    fin()

        def finalize(h, s, av):
            def f():
                ob = osb_pool.tile([HD + 1, SPAN], F32, tag="osb",
                                   name=f"ob{h}_{s}")
                nc.vector.tensor_copy(ob[:], av[:])
                nc.gpsimd.dma_start(
                    o[h * (HD + 1):(h + 1) * (HD + 1),
                      s * SPAN:(s + 1) * SPAN], ob[:])
            return f

        for s in range(NS):
            njt = CPS * (s + 1)
            q0 = s * SPAN

            # ---- h01 pass ----
            drain_until((s, 0))
            av0 = ps_av.tile([HD + 1, SPAN], F32, tag="av", name=f"av0_{s}")
            av1 = ps_av.tile([HD + 1, SPAN], F32, tag="av", name=f"av1_{s}")
            for jt in range(njt):
                c_d = jt - CPS * s
                n0 = max(c_d, 0) * P
                sc = ps_sc.tile([P, 2 * SPAN], F32, tag="sc",
                                name=f"sc_{s}_{jt}")
                nc.tensor.matmul(sc[:, n0:SPAN],
                                 kTz[0][:, jt * P:(jt + 1) * P],
                                 qT01[:, q0 + n0:q0 + SPAN],
                                 start=True, stop=True)
                nc.tensor.matmul(sc[:, SPAN + n0:2 * SPAN],
                                 kTz[1][:, jt * P:(jt + 1) * P],
                                 qT01[:, q0 + n0:q0 + SPAN],
                                 start=True, stop=True)
                pop_filler()
                if 0 <= c_d <= 1:
                    pop_filler()
                p = ppool.tile([P, 2 * SPAN], F32R, tag="p", name=f"p_{s}_{jt}")
                if n0 == 0:
                    nc.scalar.activation(p[:], sc[:], EXP)
                else:
                    sc3 = sc[:].rearrange("q (t c) -> q t c", c=SPAN)
                    p3 = p[:].rearrange("q (t c) -> q t c", c=SPAN)
                    nc.scalar.activation(p3[:, :, n0:SPAN], sc3[:, :, n0:SPAN],
                                         EXP)
                if c_d >= 0:
                    nc.vector.tensor_mul(
                        p[:, n0:n0 + P], p[:, n0:n0 + P], trimask)
                    nc.vector.tensor_mul(
                        p[:, SPAN + n0:SPAN + n0 + P],
                        p[:, SPAN + n0:SPAN + n0 + P], trimask)

                def av_emit(jt=jt, n0=n0, p=p, av0=av0, av1=av1, njt=njt):
                    nc.tensor.matmul(av0[:, n0:SPAN], vnat_lhsT(0, jt),
                                     p[:, n0:SPAN],
                                     start=(jt == 0), stop=(jt == njt - 1))
                    nc.tensor.matmul(av1[:, n0:SPAN], vnat_lhsT(1, jt),
                                     p[:, SPAN + n0:2 * SPAN],
                                     start=(jt == 0), stop=(jt == njt - 1))
                fin = None
                if jt == njt - 1:
                    f0, f1 = finalize(0, s, av0), finalize(1, s, av1)
                    fin = lambda f0=f0, f1=f1: (f0(), f1())
                avq.append((av_emit, fin))
                pump()

            # ---- h2 pass ----
            drain_until((s, 1))
            av2 = ps_av.tile([HD + 1, SPAN], F32, tag="av", name=f"av2_{s}")
            for jt in range(njt):
                c_d = jt - CPS * s
                n0 = max(c_d, 0) * P
                sc = ps_sc.tile([P, 2 * SPAN], F32, tag="sc",
                                name=f"sc2_{s}_{jt}")
                nc.tensor.matmul(sc[:, n0:SPAN],
                                 kTz[2][:, jt * P:(jt + 1) * P],
                                 qT2z[:, q0 + n0:q0 + SPAN],
                                 start=True, stop=True)
                pop_filler()
                if 0 <= c_d <= 1:
                    pop_filler()
                p = ppool.tile([P, 2 * SPAN], F32R, tag="p",
                               name=f"p2_{s}_{jt}")
                nc.scalar.activation(p[:, n0:SPAN], sc[:, n0:SPAN], EXP)
                if c_d >= 0:
                    nc.vector.tensor_mul(
                        p[:, n0:n0 + P], p[:, n0:n0 + P], trimask)

                def av_emit(jt=jt, n0=n0, p=p, av2=av2, njt=njt):
                    nc.tensor.matmul(av2[:, n0:SPAN], vnat_lhsT(2, jt),
                                     p[:, n0:SPAN],
                                     start=(jt == 0), stop=(jt == njt - 1))
                fin = finalize(2, s, av2) if jt == njt - 1 else None
                avq.append((av_emit, fin))
                pump()

        while fq:
            u = fq.popleft()
            if not (isinstance(u, tuple) and u[0] == "barrier"):
                u()
        pump(force=True)


_NC_CACHE = {}


def _get_module():
    key = "v12"
    if key not in _NC_CACHE:
        nc = bacc.Bacc("TRN2", target_bir_lowering=False, debug=False)
        with tile.TileContext(nc) as tc:
            _build(nc, tc)
        nc.compile()
        _NC_CACHE[key] = nc
    return _NC_CACHE[key]



def _make_cz():
    czv = np.zeros((P, CZ_W), np.float32)
    czv[:, 0:P] = np.eye(P, dtype=np.float32)
    czv[:, P:2 * P] = np.triu(np.ones((P, P), np.float32))
    czv[:, 2 * P:CZ_W] = 1.0
    return czv


_CZ = _make_cz()


def _in_maps(x, Wq, Wk, Wv):
    maps = []
    xT = [np.ascontiguousarray(
        x[b].T.reshape(KT, P, NS, SPAN).transpose(1, 2, 0, 3).reshape(P, -1))
        for b in range(B)]
    WqT, WkT, WvT = Wq.T, Wk.T, Wv.T
    for c in range(N_CORES):
        bc, g = divmod(c, N_CORES // B)
        s0 = g * DL
        wcomb = np.concatenate([
            WqT[:, s0:s0 + P], WkT[:, s0:s0 + P], WvT[:, s0:s0 + P],
            WkT[:, s0 + P:s0 + DL], WvT[:, s0 + P:s0 + DL],
            WqT[:, s0 + P:s0 + DL]], axis=1)
        wpk = np.ascontiguousarray(
            wcomb.reshape(KT, P, WCW).transpose(1, 0, 2).reshape(P, -1))
        maps.append({
            "xt": xT[bc],
            "wc": wpk,
            "cz": _CZ,
        })
    return maps


def kernel(x, Wq, Wk, Wv, _trace=False, _tmpdir=None, **_kw):
    x = np.asarray(x, dtype=np.float32)
    Wq = np.asarray(Wq, dtype=np.float32)
    Wk = np.asarray(Wk, dtype=np.float32)
    Wv = np.asarray(Wv, dtype=np.float32)
    assert x.shape == (B, N, D) and Wq.shape == (D, D)

    nc = _get_module()
    res = bass_utils.run_bass_kernel_spmd(
        nc, _in_maps(x, Wq, Wk, Wv), core_ids=list(range(N_CORES)),
        trace=_trace, tmpdir=_tmpdir)
    out = np.empty((B, N, D), np.float32)
    for c in range(N_CORES):
        bc, g = divmod(c, N_CORES // B)
        oT = res.results[c]["o"].astype(np.float64)
        for h in range(HL):
            blk = oT[h * (HD + 1):h * (HD + 1) + HD, :]
            den = oT[h * (HD + 1) + HD, :]
            out[bc, :, g * DL + h * HD:g * DL + (h + 1) * HD] = \
                (blk / den).T.astype(np.float32)
    if _trace:
        return out, res
    return out
